# revision 11
# baseline (speedup 1.0000x reference)
"""Trainium2 Bass kernel (v2) for nn_AggregationAndDiscriminationLoss.

Data-parallel over batch: 2 images per core on 8 cores; host applies the
tiny 16-label loss formulas to per-label sums decoded from each core.

Per image (N = 896*896 pixels as [128, 6272]), per chunk of CF=784 cols:
  - ScalarE: sq_c = sim_c^2 (4 Square ops), tkb = bf16(labels) (1 copy),
    and the label-16 mask as a single saturated-sigmoid step
    sigma(40*(x-15.5)) -- exact-to-bf16 for integer labels (top label
    needs no upper bound).
  - VectorE: tkm = [T==K]; 15 is_equal mask ops over the [T|K] tile
    (tensor_scalar 4x perf mode, ~440 cyc each -- the critical path);
    s01/s23/vs adds and vtk = vs*tkm assembled straight into the padded
    stationary tile vt[blk] = [v(28) | vtk(28) | 1] (58-stride, 4B
    aligned so tensor_tensor keeps 2x mode).
  - TensorE, 2-group column tiling: group g = blk%2 owns PE columns /
    PSUM partitions 64g..64g+56; the two groups run their matmul streams
    CONCURRENTLY (~2x).  Two 224-col matmuls per (blk, map) (8 labels x
    28 cols) let the PE start while the DVE mask burst is still running.
    PSUM po[128, 2, 512] f32 keeps each map in its own 2KB bank (a
    448-wide layout straddles the bank boundary = garbage), and only the
    first matmul into a bank carries start=True (start clears the whole
    bank).
  - per image: one ACT evac po[0:121] -> so, DMA out [121, 2, 448] f32.

Host decodes the block-diagonal entries (A, Bk, Cc, cT, cK per label) in
f64.  Measured ~157-172 us/core (baseline 236 us); engine budget: DVE
~140, ACT ~112, DMA ~108-114, PE ~80 (hidden).
"""

import numpy as np

import concourse.bass as bass
import concourse.tile as tile
from concourse import mybir
from concourse.bass_utils import run_bass_kernel_spmd

B, C, H, W = 16, 4, 896, 896
NCORES = 8
IMGS = B // NCORES
P = 128
NFREE = (H * W) // P        # 6272
NCH = 8                     # chunks per image
CF = NFREE // NCH           # 784
BC = 28                     # value-block width
NB = CF // BC               # 28 blocks per chunk
M = 2 * BC + 1              # 57 stationary cols: v | vtk | ones
MP = 58                     # padded block stride (4B-aligned strides)
K_MAX = 16
ACT_LABELS = (16,)          # labels whose masks are built on ScalarE
SIGMA_AGG = 0.5
SIGMA_DIS = 3.0


def _legalize_sync(nc):
    """Split >1-wait instructions: this walrus only encodes one sync wait."""
    for fn in nc.m.functions:
        for blk in fn.blocks:
            new = []
            for ins in blk.instructions:
                si = ins.sync_info
                if si is not None and len(si.on_wait) > 1:
                    waits = list(si.on_wait)
                    for k, w in enumerate(waits[:-1]):
                        nop = mybir.InstNoOp(name=f"{ins.name}-ws{k}", ins=[], outs=[])
                        nop.engine = ins.engine
                        nop.sync_info = mybir.SyncInfo(on_wait=[w], on_update=[])
                        new.append(nop)
                    ins.sync_info = mybir.SyncInfo(
                        on_wait=[waits[-1]], on_update=list(si.on_update)
                    )
                new.append(ins)
            blk.instructions = new


def _build_nc(reps=1, ablate=(), ch_bufs=2, mm_halves=2, act_labels=None,
              sig16=True, ge15=False, tmp_bufs=2):
    ACT_L = ACT_LABELS if act_labels is None else act_labels
    if ge15:
        ACT_L = tuple(ACT_L) + (15,)
    nc = bass.Bass()
    dt = mybir.dt
    eq = mybir.AluOpType.is_equal
    AF = mybir.ActivationFunctionType

    sim = nc.dram_tensor("sim", [IMGS, C, P, NFREE], dt.float32, kind="ExternalInput")
    tl = nc.dram_tensor("tlab", [IMGS, P, NFREE], dt.int32, kind="ExternalInput")
    kl = nc.dram_tensor("klab", [IMGS, P, NFREE], dt.int32, kind="ExternalInput")
    acc_d = nc.dram_tensor(
        "acc", [IMGS, 121, 2, 448], dt.float32, kind="ExternalOutput"
    )

    with tile.TileContext(nc) as tc:
        with (
            tc.tile_pool(name="cst", bufs=1) as cst,
            tc.tile_pool(name="io", bufs=2) as io,
            tc.tile_pool(name="tmp", bufs=2) as tmp,
            tc.tile_pool(name="mks", bufs=2) as mks,
            tc.tile_pool(name="ps", bufs=2, space="PSUM") as ps,
        ):
            bias_t = cst.tile([P, len(ACT_L) + 1], mybir.dt.float32)
            for j, l in enumerate(ACT_L):
                if sig16 and l == K_MAX:
                    bv = -620.0          # sigma(40x-620) = [x > 15.5]
                elif ge15 and l == 15:
                    bv = -580.0          # sigma(40x-580) = [x > 14.5]
                else:
                    bv = float(-l)
                nc.vector.memset(bias_t[:, j:j + 1], bv)
            nc.vector.memset(bias_t[:, len(ACT_L):], 1.0)
            # static double-buffered stationary tiles; ones col written once
            vts = []
            for v in range(2):
                vt_s = cst.tile([P, NB, MP], mybir.dt.bfloat16, name=f"vt{v}")
                nc.vector.memset(vt_s[:, :, 2 * BC:M], 1.0)
                vts.append(vt_s)

            def _image(b):
                po = ps.tile([128, 2, 512], dt.float32, tag="po")
                for ci in range(NCH):
                    cs = slice(ci * CF, (ci + 1) * CF)
                    lab = io.tile([P, 2, CF], dt.int32, tag="lab")
                    nc.sync.dma_start(lab[:, 0, :], tl[b, :, cs])
                    nc.sync.dma_start(lab[:, 1, :], kl[b, :, cs])
                    chs = []
                    for c in range(C):
                        ch = io.tile([P, CF], dt.float32, tag=f"ch{c}", bufs=ch_bufs)
                        nc.sync.dma_start(ch[:], sim[b, c, :, cs])
                        chs.append(ch)

                    # --- ScalarE: tkb first (feeds the DVE mask burst) ---
                    tkb = tmp.tile([P, 2, CF], dt.bfloat16, tag="tkb", bufs=tmp_bufs)
                    if "act" not in ablate:
                        nc.scalar.copy(tkb[:], lab[:])
                    elif "dve" not in ablate or "masks" not in ablate:
                        nc.vector.memset(tkb[:, 0, 0:1], 0.0)

                    mk = mks.tile([P, 16, 2, CF], dt.bfloat16, tag="mk")
                    if "pe" not in ablate and (
                        "masks" in ablate or "act" in ablate
                    ):
                        nc.vector.memset(mk[:, 0, 0, 0:1], 0.0)
                    ats = []
                    if "masks" not in ablate and "act" not in ablate:
                        # exact one-hot for integer labels: relu(1-2*(x-i)^2);
                        # for the top label (16) a single saturated-sigmoid
                        # step [x > 15.5] suffices (sigma(+-20) rounds to
                        # exactly 1/0-ish in bf16; the 2e-9 tail is far below
                        # the loss tolerance).
                        for j, l in enumerate(ACT_L):
                            if (sig16 and l == K_MAX) or (ge15 and l == 15):
                                # cumulative step mask; slot 15 is
                                # [x>=15] = A15+A16, telescoped on host
                                nc.scalar.activation(
                                    mk[:, l - 1, :, :], tkb[:], AF.Sigmoid,
                                    bias=bias_t[:, j:j + 1], scale=40.0,
                                )
                                ats.append(None)
                                continue
                            at = tmp.tile([P, 2, CF], dt.bfloat16, tag=f"at{j}")
                            nc.scalar.activation(
                                at[:], tkb[:], AF.Square,
                                bias=bias_t[:, j:j + 1],
                            )
                            ats.append(at)
                    sqs = []
                    for c in range(C):
                        sq = tmp.tile([P, CF], dt.bfloat16, tag=f"sq{c}", bufs=tmp_bufs)
                        if "act" not in ablate:
                            nc.scalar.activation(sq[:], chs[c][:], AF.Square)
                        elif "dve" not in ablate:
                            nc.vector.memset(sq[:, 0:1], 0.0)
                        sqs.append(sq)
                    if "masks" not in ablate and "act" not in ablate:
                        for j, l in enumerate(ACT_L):
                            if ats[j] is None:
                                continue
                            # scale=-2: tolerates ACT Square's 1-ULP error
                            # at at==1 (neighbor labels) while staying exact
                            # at at==0 (the matching label).
                            nc.scalar.activation(
                                mk[:, l - 1, :, :], ats[j][:], AF.Relu,
                                bias=bias_t[:, len(ACT_L):], scale=-2.0,
                            )

                    # --- VectorE: TSS mask burst first, then the TT block
                    # (tkm/adds/muls) -- op-type switches on the DVE flush
                    # the uop pipe, so keep each type contiguous ---
                    vt = vts[ci % 2]
                    if "masks" not in ablate:
                        for l in range(1, K_MAX + 1):
                            if l in ACT_L and "act" not in ablate:
                                continue
                            nc.vector.tensor_single_scalar(
                                mk[:, l - 1, :, :], tkb[:], float(l), eq
                            )
                    if "dve" not in ablate:
                        tkm = tmp.tile([P, CF], dt.bfloat16, tag="tkm", bufs=tmp_bufs)
                        nc.vector.tensor_tensor(
                            tkm[:], tkb[:, 0, :], tkb[:, 1, :], eq
                        )
                        s01 = tmp.tile([P, CF], dt.bfloat16, tag="s01", bufs=tmp_bufs)
                        nc.vector.tensor_add(s01[:], sqs[0][:], sqs[1][:])
                        s23 = tmp.tile([P, CF], dt.bfloat16, tag="s23", bufs=tmp_bufs)
                        nc.vector.tensor_add(s23[:], sqs[2][:], sqs[3][:])
                        vs = vt[:, :, 0:BC]
                        nc.vector.tensor_add(
                            vs,
                            s01[:].rearrange("p (b c) -> p b c", c=BC),
                            s23[:].rearrange("p (b c) -> p b c", c=BC),
                        )
                        nc.vector.tensor_mul(
                            vt[:, :, BC:2 * BC],
                            vs,
                            tkm[:].rearrange("p (b c) -> p b c", c=BC),
                        )

                    # --- TensorE: 2-group column tiling ---
                    if "pe" not in ablate:
                        for blk in range(NB):
                            g = blk % 2
                            rows = slice(64 * g, 64 * g + M)
                            lhs = vt[:, blk, 0:M]
                            bs = slice(blk * BC, (blk + 1) * BC)
                            nh = mm_halves
                            hl = 16 // nh
                            for m in range(2):
                                for h in range(nh):
                                    # start clears the whole PSUM bank of the
                                    # (group, map) region: only the first
                                    # matmul into that bank may set it.
                                    first = ci == 0 and blk < 2 and h == 0
                                    last = (ci == NCH - 1 and blk >= NB - 2
                                            and h == nh - 1)
                                    nc.tensor.matmul(
                                        po[rows, m, h * hl * BC:(h + 1) * hl * BC]
                                        .rearrange("q (i c) -> q i c", c=BC),
                                        lhs,
                                        mk[:, h * hl:(h + 1) * hl, m, bs],
                                        start=first, stop=last,
                                        skip_group_check=True,
                                    )
                so = tmp.tile([121, 2, 448], dt.float32, tag="so")
                if "pe" not in ablate:
                    nc.scalar.copy(so[:], po[0:121, :, 0:448])
                else:
                    nc.vector.memset(so[:, :, 0:1], 0.0)
                nc.sync.dma_start(acc_d[b], so[:])

            def _all_images():
                for b in range(IMGS):
                    _image(b)

            if reps == 1:
                _all_images()
            else:
                with tc.For_i(0, reps, 1):
                    _all_images()
    _legalize_sync(nc)
    return nc


_NC_CACHE = None


def _get_nc():
    global _NC_CACHE
    if _NC_CACHE is None:
        _NC_CACHE = _build_nc()
    return _NC_CACHE


GE15_DECODE = False


def _decode(acc):
    """acc: [IMGS, 121, 2, 448] -> per-image (A, Bk, Cc, cT, cK) each [16]."""
    out = []
    for b in range(IMGS):
        a = acc[b].astype(np.float64).reshape(121, 2, 16, BC)
        A = np.zeros(16)
        Bk = np.zeros(16)
        Cc = np.zeros(16)
        cT = np.zeros(16)
        cK = np.zeros(16)
        for g in (0, 1):
            r0 = 64 * g
            for c in range(BC):
                A += a[r0 + c, 0, :, c]
                Bk += a[r0 + c, 1, :, c]
                Cc += a[r0 + BC + c, 1, :, c]
            cT += a[r0 + 2 * BC, 0, :, :].sum(axis=1)
            cK += a[r0 + 2 * BC, 1, :, :].sum(axis=1)
        if GE15_DECODE:
            # slot 15 held the cumulative [x>=15] mask: telescope
            for arr in (A, Bk, Cc, cT, cK):
                arr[14] -= arr[15]
        out.append((A, Bk, Cc, cT, cK))
    return out


def _finalize(per_image):
    labels = np.arange(1, K_MAX + 1, dtype=np.float64)
    L_agg_tot = 0.0
    L_dis_tot = 0.0
    for A, Bk, Cc, cT, cK in per_image:
        nz = np.nonzero(cK > 0.5)[0]
        num_kernels = int(nz.max() + 1) if nz.size else 0
        valid = labels <= num_kernels

        denom = cK + 1.0
        x = A + Bk / (denom * denom) - 2.0 * Cc / denom
        pos = x > 0
        norm = np.where(pos, np.sqrt(np.where(pos, x, 1.0)), 0.0) - SIGMA_AGG
        agg_terms = np.log(norm * norm + 1.0) / (cT + 1.0)
        L_agg_tot += float(np.sum(np.where(valid, agg_terms, 0.0)))

        D = Bk / ((cK + 0.001) ** 2)
        S = D[:, None] + D[None, :]
        pair_mask = (labels[:, None] < labels[None, :]) & valid[None, :]
        pnorm = np.sqrt(np.where(pair_mask, S, 1.0))
        dnorm = SIGMA_DIS - pnorm
        dis_terms = np.log(dnorm * dnorm + 1.0)
        dis_sum = float(np.sum(np.where(pair_mask, dis_terms, 0.0)))
        if num_kernels > 1:
            nk = float(num_kernels)
            L_dis_tot += dis_sum / (nk * (nk - 1.0))
    return np.float32(L_agg_tot), np.float32(L_dis_tot)


def _in_maps(pred_similarities, text_mask_ndi_labels, kernel_mask_ndi_labels):
    sim = np.asarray(pred_similarities, dtype=np.float32).reshape(B, C, P, NFREE)
    T = np.asarray(text_mask_ndi_labels, dtype=np.int32).reshape(B, P, NFREE)
    K = np.asarray(kernel_mask_ndi_labels, dtype=np.int32).reshape(B, P, NFREE)

    in_maps = []
    for core in range(NCORES):
        s = slice(IMGS * core, IMGS * (core + 1))
        in_maps.append(
            {
                "sim": np.ascontiguousarray(sim[s]),
                "tlab": np.ascontiguousarray(T[s]),
                "klab": np.ascontiguousarray(K[s]),
            }
        )
    return in_maps


def _run(pred_similarities, text_mask_ndi_labels, kernel_mask_ndi_labels,
         trace=False):
    in_maps = _in_maps(
        pred_similarities, text_mask_ndi_labels, kernel_mask_ndi_labels
    )
    nc = _get_nc()
    res = run_bass_kernel_spmd(
        nc, in_maps, core_ids=list(range(NCORES)), trace=trace
    )
    per_image = []
    for core in range(NCORES):
        per_image.extend(_decode(res.results[core]["acc"]))
    return _finalize(per_image), res


def kernel(pred_similarities, text_mask_ndi_labels, kernel_mask_ndi_labels):
    out, _ = _run(pred_similarities, text_mask_ndi_labels, kernel_mask_ndi_labels)
    return out


# revision 15
# speedup vs baseline: 1.2262x; 1.2262x over previous
"""Trainium2 Bass kernel (v2) for nn_AggregationAndDiscriminationLoss.

Data-parallel over batch: 2 images per core on 8 cores; host applies the
tiny 16-label loss formulas to per-label sums decoded from each core.

Per image (N = 896*896 pixels as [128, 6272]), per chunk of CF=784 cols:
  - ScalarE: sq_c = sim_c^2 (4 Square ops), tkb = bf16(labels) (1 copy),
    and the label-16 mask as a single saturated-sigmoid step
    sigma(40*(x-15.5)) -- exact-to-bf16 for integer labels (top label
    needs no upper bound).
  - VectorE: tkm = [T==K]; 15 is_equal mask ops over the [T|K] tile
    (tensor_scalar 4x perf mode, ~440 cyc each -- the critical path);
    s01/s23/vs adds and vtk = vs*tkm assembled straight into the padded
    stationary tile vt[blk] = [v(28) | vtk(28) | 1] (58-stride, 4B
    aligned so tensor_tensor keeps 2x mode).
  - TensorE, 2-group column tiling: group g = blk%2 owns PE columns /
    PSUM partitions 64g..64g+56; the two groups run their matmul streams
    CONCURRENTLY (~2x).  Two 224-col matmuls per (blk, map) (8 labels x
    28 cols) let the PE start while the DVE mask burst is still running.
    PSUM po[128, 2, 512] f32 keeps each map in its own 2KB bank (a
    448-wide layout straddles the bank boundary = garbage), and only the
    first matmul into a bank carries start=True (start clears the whole
    bank).
  - per image: one ACT evac po[0:121] -> so, DMA out [121, 2, 448] f32.

Host decodes the block-diagonal entries (A, Bk, Cc, cT, cK per label) in
f64.  Label tiles triple-buffered (lab_bufs=3) so the mask-critical
label DMA runs ahead of the sim stream.  Measured ~144-160 us/core
(baseline ~230-237 us); engine budget: DVE ~140 (critical), ACT ~112,
DMA ~108-114, PE ~80 (hidden).
"""

import numpy as np

import concourse.bass as bass
import concourse.tile as tile
from concourse import mybir
from concourse.bass_utils import run_bass_kernel_spmd

B, C, H, W = 16, 4, 896, 896
NCORES = 8
IMGS = B // NCORES
P = 128
NFREE = (H * W) // P        # 6272
NCH = 8                     # chunks per image
CF = NFREE // NCH           # 784
BC = 28                     # value-block width
NB = CF // BC               # 28 blocks per chunk
M = 2 * BC + 1              # 57 stationary cols: v | vtk | ones
MP = 58                     # padded block stride (4B-aligned strides)
K_MAX = 16
ACT_LABELS = (16,)          # labels whose masks are built on ScalarE
SIGMA_AGG = 0.5
SIGMA_DIS = 3.0


def _legalize_sync(nc):
    """Split >1-wait instructions: this walrus only encodes one sync wait."""
    for fn in nc.m.functions:
        for blk in fn.blocks:
            new = []
            for ins in blk.instructions:
                si = ins.sync_info
                if si is not None and len(si.on_wait) > 1:
                    waits = list(si.on_wait)
                    for k, w in enumerate(waits[:-1]):
                        nop = mybir.InstNoOp(name=f"{ins.name}-ws{k}", ins=[], outs=[])
                        nop.engine = ins.engine
                        nop.sync_info = mybir.SyncInfo(on_wait=[w], on_update=[])
                        new.append(nop)
                    ins.sync_info = mybir.SyncInfo(
                        on_wait=[waits[-1]], on_update=list(si.on_update)
                    )
                new.append(ins)
            blk.instructions = new


def _build_nc(reps=1, ablate=(), ch_bufs=2, mm_halves=2, act_labels=None,
              sig16=True, ge15=False, tmp_bufs=2, lab_bufs=3):
    ACT_L = ACT_LABELS if act_labels is None else act_labels
    if ge15:
        ACT_L = tuple(ACT_L) + (15,)
    nc = bass.Bass()
    dt = mybir.dt
    eq = mybir.AluOpType.is_equal
    AF = mybir.ActivationFunctionType

    sim = nc.dram_tensor("sim", [IMGS, C, P, NFREE], dt.float32, kind="ExternalInput")
    tl = nc.dram_tensor("tlab", [IMGS, P, NFREE], dt.int32, kind="ExternalInput")
    kl = nc.dram_tensor("klab", [IMGS, P, NFREE], dt.int32, kind="ExternalInput")
    acc_d = nc.dram_tensor(
        "acc", [IMGS, 121, 2, 448], dt.float32, kind="ExternalOutput"
    )

    with tile.TileContext(nc) as tc:
        with (
            tc.tile_pool(name="cst", bufs=1) as cst,
            tc.tile_pool(name="io", bufs=2) as io,
            tc.tile_pool(name="tmp", bufs=2) as tmp,
            tc.tile_pool(name="mks", bufs=2) as mks,
            tc.tile_pool(name="ps", bufs=2, space="PSUM") as ps,
        ):
            bias_t = cst.tile([P, len(ACT_L) + 1], mybir.dt.float32)
            for j, l in enumerate(ACT_L):
                if sig16 and l == K_MAX:
                    bv = -620.0          # sigma(40x-620) = [x > 15.5]
                elif ge15 and l == 15:
                    bv = -580.0          # sigma(40x-580) = [x > 14.5]
                else:
                    bv = float(-l)
                nc.vector.memset(bias_t[:, j:j + 1], bv)
            nc.vector.memset(bias_t[:, len(ACT_L):], 1.0)
            # static double-buffered stationary tiles; ones col written once
            vts = []
            for v in range(2):
                vt_s = cst.tile([P, NB, MP], mybir.dt.bfloat16, name=f"vt{v}")
                nc.vector.memset(vt_s[:, :, 2 * BC:M], 1.0)
                vts.append(vt_s)

            def _image(b):
                po = ps.tile([128, 2, 512], dt.float32, tag="po")
                for ci in range(NCH):
                    cs = slice(ci * CF, (ci + 1) * CF)
                    lab = io.tile([P, 2, CF], dt.int32, tag="lab", bufs=lab_bufs)
                    nc.sync.dma_start(lab[:, 0, :], tl[b, :, cs])
                    nc.sync.dma_start(lab[:, 1, :], kl[b, :, cs])
                    chs = []
                    for c in range(C):
                        ch = io.tile([P, CF], dt.float32, tag=f"ch{c}", bufs=ch_bufs)
                        nc.sync.dma_start(ch[:], sim[b, c, :, cs])
                        chs.append(ch)

                    # --- ScalarE: tkb first (feeds the DVE mask burst) ---
                    tkb = tmp.tile([P, 2, CF], dt.bfloat16, tag="tkb", bufs=tmp_bufs)
                    if "act" not in ablate:
                        nc.scalar.copy(tkb[:], lab[:])
                    elif "dve" not in ablate or "masks" not in ablate:
                        nc.vector.memset(tkb[:, 0, 0:1], 0.0)

                    mk = mks.tile([P, 16, 2, CF], dt.bfloat16, tag="mk")
                    if "pe" not in ablate and (
                        "masks" in ablate or "act" in ablate
                    ):
                        nc.vector.memset(mk[:, 0, 0, 0:1], 0.0)
                    ats = []
                    if "masks" not in ablate and "act" not in ablate:
                        # exact one-hot for integer labels: relu(1-2*(x-i)^2);
                        # for the top label (16) a single saturated-sigmoid
                        # step [x > 15.5] suffices (sigma(+-20) rounds to
                        # exactly 1/0-ish in bf16; the 2e-9 tail is far below
                        # the loss tolerance).
                        for j, l in enumerate(ACT_L):
                            if (sig16 and l == K_MAX) or (ge15 and l == 15):
                                # cumulative step mask; slot 15 is
                                # [x>=15] = A15+A16, telescoped on host
                                nc.scalar.activation(
                                    mk[:, l - 1, :, :], tkb[:], AF.Sigmoid,
                                    bias=bias_t[:, j:j + 1], scale=40.0,
                                )
                                ats.append(None)
                                continue
                            at = tmp.tile([P, 2, CF], dt.bfloat16, tag=f"at{j}")
                            nc.scalar.activation(
                                at[:], tkb[:], AF.Square,
                                bias=bias_t[:, j:j + 1],
                            )
                            ats.append(at)
                    sqs = []
                    for c in range(C):
                        sq = tmp.tile([P, CF], dt.bfloat16, tag=f"sq{c}", bufs=tmp_bufs)
                        if "act" not in ablate:
                            nc.scalar.activation(sq[:], chs[c][:], AF.Square)
                        elif "dve" not in ablate:
                            nc.vector.memset(sq[:, 0:1], 0.0)
                        sqs.append(sq)
                    if "masks" not in ablate and "act" not in ablate:
                        for j, l in enumerate(ACT_L):
                            if ats[j] is None:
                                continue
                            # scale=-2: tolerates ACT Square's 1-ULP error
                            # at at==1 (neighbor labels) while staying exact
                            # at at==0 (the matching label).
                            nc.scalar.activation(
                                mk[:, l - 1, :, :], ats[j][:], AF.Relu,
                                bias=bias_t[:, len(ACT_L):], scale=-2.0,
                            )

                    # --- VectorE: TSS mask burst first, then the TT block
                    # (tkm/adds/muls) -- op-type switches on the DVE flush
                    # the uop pipe, so keep each type contiguous ---
                    vt = vts[ci % 2]
                    if "masks" not in ablate:
                        for l in range(1, K_MAX + 1):
                            if l in ACT_L and "act" not in ablate:
                                continue
                            nc.vector.tensor_single_scalar(
                                mk[:, l - 1, :, :], tkb[:], float(l), eq
                            )
                    if "dve" not in ablate:
                        tkm = tmp.tile([P, CF], dt.bfloat16, tag="tkm", bufs=tmp_bufs)
                        nc.vector.tensor_tensor(
                            tkm[:], tkb[:, 0, :], tkb[:, 1, :], eq
                        )
                        s01 = tmp.tile([P, CF], dt.bfloat16, tag="s01", bufs=tmp_bufs)
                        nc.vector.tensor_add(s01[:], sqs[0][:], sqs[1][:])
                        s23 = tmp.tile([P, CF], dt.bfloat16, tag="s23", bufs=tmp_bufs)
                        nc.vector.tensor_add(s23[:], sqs[2][:], sqs[3][:])
                        vs = vt[:, :, 0:BC]
                        nc.vector.tensor_add(
                            vs,
                            s01[:].rearrange("p (b c) -> p b c", c=BC),
                            s23[:].rearrange("p (b c) -> p b c", c=BC),
                        )
                        nc.vector.tensor_mul(
                            vt[:, :, BC:2 * BC],
                            vs,
                            tkm[:].rearrange("p (b c) -> p b c", c=BC),
                        )

                    # --- TensorE: 2-group column tiling ---
                    if "pe" not in ablate:
                        for blk in range(NB):
                            g = blk % 2
                            rows = slice(64 * g, 64 * g + M)
                            lhs = vt[:, blk, 0:M]
                            bs = slice(blk * BC, (blk + 1) * BC)
                            nh = mm_halves
                            hl = 16 // nh
                            for m in range(2):
                                for h in range(nh):
                                    # start clears the whole PSUM bank of the
                                    # (group, map) region: only the first
                                    # matmul into that bank may set it.
                                    first = ci == 0 and blk < 2 and h == 0
                                    last = (ci == NCH - 1 and blk >= NB - 2
                                            and h == nh - 1)
                                    nc.tensor.matmul(
                                        po[rows, m, h * hl * BC:(h + 1) * hl * BC]
                                        .rearrange("q (i c) -> q i c", c=BC),
                                        lhs,
                                        mk[:, h * hl:(h + 1) * hl, m, bs],
                                        start=first, stop=last,
                                        skip_group_check=True,
                                    )
                so = tmp.tile([121, 2, 448], dt.float32, tag="so")
                if "pe" not in ablate:
                    nc.scalar.copy(so[:], po[0:121, :, 0:448])
                else:
                    nc.vector.memset(so[:, :, 0:1], 0.0)
                nc.sync.dma_start(acc_d[b], so[:])

            def _all_images():
                for b in range(IMGS):
                    _image(b)

            if reps == 1:
                _all_images()
            else:
                with tc.For_i(0, reps, 1):
                    _all_images()
    _legalize_sync(nc)
    return nc


_NC_CACHE = None


def _get_nc():
    global _NC_CACHE
    if _NC_CACHE is None:
        _NC_CACHE = _build_nc()
    return _NC_CACHE


GE15_DECODE = False


def _decode(acc):
    """acc: [IMGS, 121, 2, 448] -> per-image (A, Bk, Cc, cT, cK) each [16]."""
    out = []
    for b in range(IMGS):
        a = acc[b].astype(np.float64).reshape(121, 2, 16, BC)
        A = np.zeros(16)
        Bk = np.zeros(16)
        Cc = np.zeros(16)
        cT = np.zeros(16)
        cK = np.zeros(16)
        for g in (0, 1):
            r0 = 64 * g
            for c in range(BC):
                A += a[r0 + c, 0, :, c]
                Bk += a[r0 + c, 1, :, c]
                Cc += a[r0 + BC + c, 1, :, c]
            cT += a[r0 + 2 * BC, 0, :, :].sum(axis=1)
            cK += a[r0 + 2 * BC, 1, :, :].sum(axis=1)
        if GE15_DECODE:
            # slot 15 held the cumulative [x>=15] mask: telescope
            for arr in (A, Bk, Cc, cT, cK):
                arr[14] -= arr[15]
        out.append((A, Bk, Cc, cT, cK))
    return out


def _finalize(per_image):
    labels = np.arange(1, K_MAX + 1, dtype=np.float64)
    L_agg_tot = 0.0
    L_dis_tot = 0.0
    for A, Bk, Cc, cT, cK in per_image:
        nz = np.nonzero(cK > 0.5)[0]
        num_kernels = int(nz.max() + 1) if nz.size else 0
        valid = labels <= num_kernels

        denom = cK + 1.0
        x = A + Bk / (denom * denom) - 2.0 * Cc / denom
        pos = x > 0
        norm = np.where(pos, np.sqrt(np.where(pos, x, 1.0)), 0.0) - SIGMA_AGG
        agg_terms = np.log(norm * norm + 1.0) / (cT + 1.0)
        L_agg_tot += float(np.sum(np.where(valid, agg_terms, 0.0)))

        D = Bk / ((cK + 0.001) ** 2)
        S = D[:, None] + D[None, :]
        pair_mask = (labels[:, None] < labels[None, :]) & valid[None, :]
        pnorm = np.sqrt(np.where(pair_mask, S, 1.0))
        dnorm = SIGMA_DIS - pnorm
        dis_terms = np.log(dnorm * dnorm + 1.0)
        dis_sum = float(np.sum(np.where(pair_mask, dis_terms, 0.0)))
        if num_kernels > 1:
            nk = float(num_kernels)
            L_dis_tot += dis_sum / (nk * (nk - 1.0))
    return np.float32(L_agg_tot), np.float32(L_dis_tot)


def _in_maps(pred_similarities, text_mask_ndi_labels, kernel_mask_ndi_labels):
    sim = np.asarray(pred_similarities, dtype=np.float32).reshape(B, C, P, NFREE)
    T = np.asarray(text_mask_ndi_labels, dtype=np.int32).reshape(B, P, NFREE)
    K = np.asarray(kernel_mask_ndi_labels, dtype=np.int32).reshape(B, P, NFREE)

    in_maps = []
    for core in range(NCORES):
        s = slice(IMGS * core, IMGS * (core + 1))
        in_maps.append(
            {
                "sim": np.ascontiguousarray(sim[s]),
                "tlab": np.ascontiguousarray(T[s]),
                "klab": np.ascontiguousarray(K[s]),
            }
        )
    return in_maps


def _run(pred_similarities, text_mask_ndi_labels, kernel_mask_ndi_labels,
         trace=False):
    in_maps = _in_maps(
        pred_similarities, text_mask_ndi_labels, kernel_mask_ndi_labels
    )
    nc = _get_nc()
    res = run_bass_kernel_spmd(
        nc, in_maps, core_ids=list(range(NCORES)), trace=trace
    )
    per_image = []
    for core in range(NCORES):
        per_image.extend(_decode(res.results[core]["acc"]))
    return _finalize(per_image), res


def kernel(pred_similarities, text_mask_ndi_labels, kernel_mask_ndi_labels):
    out, _ = _run(pred_similarities, text_mask_ndi_labels, kernel_mask_ndi_labels)
    return out
